# revision 6
# baseline (speedup 1.0000x reference)
"""Trainium2 Bass kernel for Top-1 MoE (nn_MoETop1).

Expert-parallel across 8 NeuronCores: core i owns expert i. The router is
computed token-sharded (each core transposes + matmuls its 1024-token shard)
and logits are AllGathered; every core then replicates the cheap routing tail
(softmax stats, argmax, cumsum ranks) so dispatch needs no data-dependent
communication. Dispatch = indirect scatter of token ids into a slot->token
map + one indirect row-gather of x. FFN runs in float32r (full PE rate at
free-dim >= 256). Expert outputs are AllGathered and each core row-gathers,
scales and writes its own 1024-token output shard.

Self-contained: hardcodes all shapes from the problem spec.
"""

import os

os.environ.setdefault("JAX_PLATFORMS", "")  # leave axon/neuron platform intact

import numpy as np

import concourse.bass as bass
import concourse.mybir as mybir
import concourse.tile as tile
from concourse import bacc
from concourse.bass import ds, ts
from concourse.masks import make_identity, make_upper_triangular

P = 128          # partitions
E = 8            # experts == cores
D = 1024         # d_model
H = 4096         # d_hidden
T = 8192         # tokens
TS = T // E      # tokens per shard (1024)
B = T // P       # free positions per partition in router layout (64)
CAP = 1280       # capacity = int(1.25 * T / E)
G = CAP // P     # capacity tiles (10)
KD = D // P      # d k-tiles (8)
MH = H // P      # h tiles (32)
DDN = D // P     # output d tiles (8)
NCH = 256        # FFN c-chunk (psum-bank friendly, fp32r full rate)
NN = CAP // NCH  # 5 c-chunks
MG = 4           # h-tiles per FFN group
NGRP = MH // MG  # 8 groups
BIG = float(1 << 20)
AUX_SCALE = float(E) * 0.01 / (T * T)

F32 = mybir.dt.float32
F32R = mybir.dt.float32r
I32 = mybir.dt.int32


def _emit(tc, nc, io):
    v = nc.vector
    sc = nc.scalar
    te = nc.tensor
    gp = nc.gpsimd
    sy = nc.sync
    A = mybir.AluOpType
    AF = mybir.ActivationFunctionType
    RG = [list(range(E))]

    x_d = io["x"]; xsh_d = io["xsh"]; rwt_d = io["rwt"]; rbb_d = io["rbb"]
    w1_d = io["w1h"]; b1_d = io["b1c"]; w2_d = io["w2h"]; b2_d = io["b2c"]
    mye_d = io["mye"]; rid_d = io["rowids"]
    y_d = io["y_shard"]; aux_d = io["aux"]; cnt_d = io["counts"]

    ctx = io["ctx"]
    pconst = ctx.enter_context(tc.tile_pool(name="pconst", bufs=1))
    pio = ctx.enter_context(tc.tile_pool(name="pio", bufs=3))
    pxt = ctx.enter_context(tc.tile_pool(name="pxt", bufs=2))
    prt = ctx.enter_context(tc.tile_pool(name="prt", bufs=1))
    psmall = ctx.enter_context(tc.tile_pool(name="psmall", bufs=2))
    pbig = ctx.enter_context(tc.tile_pool(name="pbig", bufs=1))
    pw1 = ctx.enter_context(tc.tile_pool(name="pw1", bufs=3))
    pw2 = ctx.enter_context(tc.tile_pool(name="pw2", bufs=MG + 1))
    pps1 = ctx.enter_context(tc.tile_pool(name="pps1", bufs=2, space="PSUM"))
    ppsh = ctx.enter_context(tc.tile_pool(name="ppsh", bufs=2, space="PSUM"))
    pps2 = ctx.enter_context(tc.tile_pool(name="pps2", bufs=1, space="PSUM"))
    pdram = ctx.enter_context(tc.tile_pool(name="pdram", bufs=1, space="DRAM"))

    # ---- internal DRAM ----
    lg_in = pdram.tile([TS, E], F32)
    lg_out = pdram.tile([T, E], F32, addr_space="Shared")
    inv_dr = pdram.tile([CAP, 1], I32)
    scomb_dr = pdram.tile([T // E, E], I32)
    scale_dr = pdram.tile([T // E, E], F32)
    eo_in = pdram.tile([CAP, D], F32)
    eo_out = pdram.tile([E * CAP, D], F32, addr_space="Shared")

    # ---- constants ----
    ident = pconst.tile([P, P], F32)
    make_identity(nc, ident[:])
    U = pconst.tile([P, P], F32)
    make_upper_triangular(nc, U[:], val=1.0, diag=False)
    iota_i = pconst.tile([P, E], I32)
    gp.iota(iota_i[:], pattern=[[1, E]], base=0, channel_multiplier=0)
    iota_f = pconst.tile([P, E], F32)
    v.tensor_copy(out=iota_f[:], in_=iota_i[:])
    e8m = pconst.tile([P, E], F32)
    v.tensor_scalar(out=e8m[:], in0=iota_f[:], scalar1=-1.0, scalar2=float(E),
                    op0=A.mult, op1=A.add)
    tokid = pconst.tile([P, B], I32)
    gp.iota(tokid[:], pattern=[[1, B]], base=0, channel_multiplier=B)
    ones = pconst.tile([P, 1], F32)
    v.memset(ones[:], 1.0)
    rwt = pconst.tile([P, KD * E], F32)
    sy.dma_start(out=rwt[:], in_=rwt_d[:, :])
    rbb = pconst.tile([P, E], F32)
    sy.dma_start(out=rbb[:], in_=rbb_d[:, :])
    mye = pconst.tile([P, 1], F32)
    sy.dma_start(out=mye[:], in_=mye_d[:, :])
    b1c = pconst.tile([P, MH], F32)
    sy.dma_start(out=b1c[:], in_=b1_d[:, :])
    b2c = pconst.tile([P, DDN], F32)
    sy.dma_start(out=b2c[:], in_=b2_d[:, :])
    rids = pconst.tile([P, 1], I32)
    sy.dma_start(out=rids[:], in_=rid_d[:, :])

    # ---- router: logits for own shard, then AllGather ----
    for tt in range(TS // P):
        xsh_t = pio.tile([P, D], F32, tag="io1024")
        sy.dma_start(out=xsh_t[:], in_=xsh_d[ts(tt, P), :])
        xt_t = pxt.tile([P, D], F32, tag="xt")
        for k in range(KD):
            pst = pps1.tile([P, P], F32, space="PSUM", tag="ps1")
            te.transpose(out=pst[:], in_=xsh_t[:, ts(k, P)], identity=ident[:])
            v.tensor_copy(out=xt_t[:, ts(k, P)], in_=pst[:])
        psl = pps1.tile([P, E], F32, space="PSUM", tag="ps1")
        for k in range(KD):
            te.matmul(out=psl[:], lhsT=xt_t[:, ts(k, P)], rhs=rwt[:, ts(k, E)],
                      start=(k == 0), stop=(k == KD - 1))
        lg_sb = psmall.tile([P, E], F32, tag="lgsb")
        v.tensor_copy(out=lg_sb[:], in_=psl[:])
        sy.dma_start(out=lg_in[ts(tt, P), :], in_=lg_sb[:])

    gp.collective_compute(
        "AllGather", A.bypass, replica_groups=RG,
        ins=[lg_in[:].opt()], outs=[lg_out[:].opt()],
    )

    # ---- router tail (replicated on all cores) ----
    # b-major layout [p, b, e], token t = p*B + b
    lgf = prt.tile([P, B * E], F32)
    sy.dma_start(out=lgf[:], in_=lg_out[:, :].rearrange("(p b) e -> p (b e)", p=P))
    lgf3 = lgf[:].rearrange("p (b e) -> p b e", e=E)
    v.tensor_tensor(out=lgf3, in0=lgf3, in1=rbb[:, None, :].to_broadcast((P, B, E)),
                    op=A.add)
    lmax = prt.tile([P, B], F32)
    v.reduce_max(out=lmax[:], in_=lgf3, axis=mybir.AxisListType.X)
    lsub = prt.tile([P, B * E], F32)
    lsub3 = lsub[:].rearrange("p (b e) -> p b e", e=E)
    v.tensor_tensor(out=lsub3, in0=lgf3,
                    in1=lmax[:, :, None].to_broadcast((P, B, E)), op=A.subtract)
    eexp = prt.tile([P, B * E], F32)
    eexp3 = eexp[:].rearrange("p (b e) -> p b e", e=E)
    sc.activation(out=eexp[:], in_=lsub[:], func=AF.Exp)
    sums = prt.tile([P, B], F32)
    v.reduce_sum(out=sums[:], in_=eexp3, axis=mybir.AxisListType.X)
    tv = prt.tile([P, B], F32)
    v.reciprocal(out=tv[:], in_=sums[:])
    gt = prt.tile([P, B * E], F32)
    gt3 = gt[:].rearrange("p (b e) -> p b e", e=E)
    v.tensor_tensor(out=gt3, in0=eexp3, in1=tv[:, :, None].to_broadcast((P, B, E)),
                    op=A.mult)
    gbsum = prt.tile([P, E], F32)
    v.reduce_sum(out=gbsum[:], in_=gt3.rearrange("p b e -> p e b"),
                 axis=mybir.AxisListType.X)
    # argmax (first max index) via 8-e trick; exact fp32 compares
    iseq = prt.tile([P, B * E], F32)
    iseq3 = iseq[:].rearrange("p (b e) -> p b e", e=E)
    v.tensor_tensor(out=iseq3, in0=lgf3,
                    in1=lmax[:, :, None].to_broadcast((P, B, E)), op=A.is_equal)
    v.tensor_tensor(out=iseq3, in0=iseq3,
                    in1=e8m[:, None, :].to_broadcast((P, B, E)), op=A.mult)
    mx8 = prt.tile([P, B], F32)
    v.reduce_max(out=mx8[:], in_=iseq3, axis=mybir.AxisListType.X)
    top = prt.tile([P, B], F32)
    v.tensor_scalar(out=top[:], in0=mx8[:], scalar1=-1.0, scalar2=float(E),
                    op0=A.mult, op1=A.add)
    # one_hot e-major [p, e, b] + flat scan + carry fix = global cumsum ranks
    ohm = prt.tile([P, E * B], F32)
    ohm3 = ohm[:].rearrange("p (e b) -> p e b", b=B)
    v.tensor_tensor(out=ohm3, in0=iota_f[:, :, None].to_broadcast((P, E, B)),
                    in1=top[:, None, :].to_broadcast((P, E, B)), op=A.is_equal)
    scan = prt.tile([P, E * B], F32)
    v.tensor_tensor_scan(out=scan[:], data0=ohm[:], data1=ohm[:], initial=0.0,
                         op0=A.add, op1=A.bypass)
    scan3 = scan[:].rearrange("p (e b) -> p e b", b=B)
    carry = prt.tile([P, E], F32)
    v.memset(carry[:, 0:1], 0.0)
    v.tensor_copy(out=carry[:, 1:E], in_=scan3[:, 0:E - 1, B - 1])
    total = prt.tile([P, E], F32)
    v.tensor_sub(out=total[:], in0=scan3[:, :, B - 1], in1=carry[:])
    pos = prt.tile([P, E * B], F32)
    pos3 = pos[:].rearrange("p (e b) -> p e b", b=B)
    v.tensor_tensor(out=pos3, in0=scan3,
                    in1=carry[:, :, None].to_broadcast((P, E, B)), op=A.subtract)
    v.tensor_sub(out=pos3, in0=pos3, in1=ohm3)
    ppre = pps1.tile([P, E], F32, space="PSUM", tag="ps1")
    te.matmul(out=ppre[:], lhsT=U[:], rhs=total[:], start=True, stop=True)
    v.tensor_tensor(out=pos3, in0=pos3,
                    in1=ppre[:, :, None].to_broadcast((P, E, B)), op=A.add)
    # aux loss + used counts
    cntp = pps1.tile([1, E], F32, space="PSUM", tag="ps1")
    te.matmul(out=cntp[:], lhsT=ones[:], rhs=total[:], start=True, stop=True)
    impp = pps1.tile([1, E], F32, space="PSUM", tag="ps1")
    te.matmul(out=impp[:], lhsT=ones[:], rhs=gbsum[:], start=True, stop=True)
    cnts = psmall.tile([1, E], F32, tag="cnts")
    v.tensor_copy(out=cnts[:], in_=cntp[:])
    cnti = psmall.tile([1, E], I32, tag="cnti")
    cntm = psmall.tile([1, E], F32, tag="cntm")
    v.tensor_scalar_min(out=cntm[:], in0=cnts[:], scalar1=float(CAP))
    v.tensor_copy(out=cnti[:], in_=cntm[:])
    sy.dma_start(out=cnt_d[:, :], in_=cnti[:])
    prod = psmall.tile([1, E], F32, tag="prod")
    v.tensor_tensor(out=prod[:], in0=impp[:], in1=cnts[:], op=A.mult)
    auxs = psmall.tile([1, 1], F32, tag="auxs")
    v.reduce_sum(out=auxs[:], in_=prod[:], axis=mybir.AxisListType.X)
    v.tensor_scalar_mul(out=auxs[:], in0=auxs[:], scalar1=AUX_SCALE)
    sy.dma_start(out=aux_d[:, :], in_=auxs[:])
    # rank / keep / scale / scatter+combine indices
    tbm = prt.tile([P, B * E], F32)
    tbm3 = tbm[:].rearrange("p (b e) -> p b e", e=E)
    v.tensor_tensor(out=tbm3, in0=pos3.rearrange("p e b -> p b e"),
                    in1=ohm3.rearrange("p e b -> p b e"), op=A.mult)
    rank = prt.tile([P, B], F32)
    v.reduce_sum(out=rank[:], in_=tbm3, axis=mybir.AxisListType.X)
    keep = prt.tile([P, B], F32)
    v.tensor_scalar(out=keep[:], in0=rank[:], scalar1=float(CAP), scalar2=None,
                    op0=A.is_lt)
    scl = prt.tile([P, B], F32)
    v.tensor_tensor(out=scl[:], in0=tv[:], in1=keep[:], op=A.mult)
    rkk = prt.tile([P, B], F32)
    v.tensor_tensor(out=rkk[:], in0=rank[:], in1=keep[:], op=A.mult)
    scf = prt.tile([P, B], F32)
    v.scalar_tensor_tensor(out=scf[:], in0=top[:], scalar=float(CAP), in1=rkk[:],
                           op0=A.mult, op1=A.add)
    sci = prt.tile([P, B], I32)
    v.tensor_copy(out=sci[:], in_=scf[:])
    sy.dma_start(out=scomb_dr[:, :].rearrange("(p r) e -> p (r e)", p=P), in_=sci[:])
    sy.dma_start(out=scale_dr[:, :].rearrange("(p r) e -> p (r e)", p=P), in_=scl[:])
    own = prt.tile([P, B], F32)
    v.tensor_tensor(out=own[:], in0=top[:], in1=mye[:, 0:1].to_broadcast((P, B)),
                    op=A.is_equal)
    both = prt.tile([P, B], F32)
    v.tensor_tensor(out=both[:], in0=own[:], in1=keep[:], op=A.mult)
    sof = prt.tile([P, B], F32)
    v.scalar_tensor_tensor(out=sof[:], in0=both[:], scalar=-BIG, in1=rank[:],
                           op0=A.mult, op1=A.add)
    v.tensor_scalar_add(out=sof[:], in0=sof[:], scalar1=BIG)
    soi = prt.tile([P, B], I32)
    v.tensor_copy(out=soi[:], in_=sof[:])

    # ---- dispatch: slot->token map, gather x rows, transpose to bufT ----
    zed = psmall.tile([P, G], I32, tag="zed")
    v.memset(zed[:], 0)
    sy.dma_start(out=inv_dr[:, :].rearrange("(g p) o -> p (g o)", p=P), in_=zed[:])
    for b in range(B):
        gp.indirect_dma_start(
            out=inv_dr[:], out_offset=bass.IndirectOffsetOnAxis(ap=soi[:, b:b + 1], axis=0),
            in_=tokid[:, b:b + 1], in_offset=None,
            bounds_check=CAP - 1, oob_is_err=False,
        )
    invs = pconst.tile([P, G], I32)
    sy.dma_start(out=invs[:], in_=inv_dr[:, :].rearrange("(g p) o -> p (g o)", p=P))

    bufT = pbig.tile([P, KD * CAP], F32R, tag="bufT")
    for g in range(G):
        bufg = pio.tile([P, D], F32, tag="io1024")
        gp.indirect_dma_start(
            out=bufg[:], out_offset=None, in_=x_d[:, :],
            in_offset=bass.IndirectOffsetOnAxis(ap=invs[:, g:g + 1], axis=0),
        )
        for k in range(KD):
            pst = pps1.tile([P, P], F32, space="PSUM", tag="ps1")
            te.transpose(out=pst[:], in_=bufg[:, ts(k, P)], identity=ident[:])
            v.tensor_copy(out=bufT[:, k * CAP + g * P: k * CAP + (g + 1) * P],
                          in_=pst[:])

    # ---- FFN: h = relu(bufT.T @ w1 + b1); out = h.T @ w2 + b2 (fp32r) ----
    acc = pbig.tile([P, DDN * CAP], F32, tag="bigacc")
    hact = pbig.tile([P, MG * CAP], F32R, tag="hact")
    for grp in range(NGRP):
        w2ts = []
        for mi in range(MG):
            m = grp * MG + mi
            w1t = pw1.tile([P, KD * P], F32R, tag="w1")
            sy.dma_start(out=w1t[:], in_=w1_d[ts(m, P), :])
            w2t = pw2.tile([P, DDN * P], F32R, tag="w2")
            sy.dma_start(out=w2t[:], in_=w2_d[ts(m, P), :])
            w2ts.append(w2t)
            for n in range(NN):
                psh = ppsh.tile([P, NCH], F32, space="PSUM", tag="psh")
                for k in range(KD):
                    te.matmul(
                        out=psh[:],
                        lhsT=w1t[:, ts(k, P)],
                        rhs=bufT[:, k * CAP + n * NCH: k * CAP + (n + 1) * NCH],
                        start=(k == 0), stop=(k == KD - 1),
                    )
                sc.activation(out=hact[:, mi * CAP + n * NCH: mi * CAP + (n + 1) * NCH],
                              in_=psh[:], func=AF.Relu, bias=b1c[:, m:m + 1])
        for n in range(NN):
            ps2 = pps2.tile([P, DDN * NCH], F32, space="PSUM", tag="ps2")
            for dd in range(DDN):
                for mi in range(MG):
                    te.matmul(
                        out=ps2[:, ts(dd, NCH)],
                        lhsT=w2ts[mi][:, ts(dd, P)],
                        rhs=hact[:, mi * CAP + n * NCH: mi * CAP + (n + 1) * NCH],
                        start=(mi == 0), stop=(mi == MG - 1),
                    )
            # accumulate into acc, one op per psum bank (2 dd slices each)
            accv = acc[:].rearrange("p (dd c) -> p dd c", c=CAP)
            ps2v = ps2[:].rearrange("p (dd c) -> p dd c", c=NCH)
            for hb in range(DDN // 2):
                av = accv[:, 2 * hb:2 * hb + 2, n * NCH:(n + 1) * NCH]
                pv = ps2v[:, 2 * hb:2 * hb + 2, :]
                if grp == 0:
                    v.tensor_copy(out=av, in_=pv)
                else:
                    v.tensor_tensor(out=av, in0=av, in1=pv, op=A.add)

    # ---- bias2, transpose back to token-major, write expert output ----
    accv = acc[:].rearrange("p (dd c) -> p dd c", c=CAP)
    for dd in range(DDN):
        v.tensor_scalar_add(out=accv[:, dd, :], in0=accv[:, dd, :],
                            scalar1=b2c[:, dd:dd + 1])
    for g in range(G):
        otk = pio.tile([P, D], F32, tag="io1024")
        for dd in range(DDN):
            pst = pps1.tile([P, P], F32, space="PSUM", tag="ps1")
            te.transpose(out=pst[:], in_=accv[:, dd, ts(g, P)], identity=ident[:])
            v.tensor_copy(out=otk[:, ts(dd, P)], in_=pst[:])
        sy.dma_start(out=eo_in[ts(g, P), :], in_=otk[:])

    gp.collective_compute(
        "AllGather", A.bypass, replica_groups=RG,
        ins=[eo_in[:].opt()], outs=[eo_out[:].opt()],
    )

    # ---- combine: gather own tokens' rows, scale, write y shard ----
    sval = psmall.tile([P, E], I32, tag="sval")
    gp.indirect_dma_start(
        out=sval[:], out_offset=None, in_=scomb_dr[:, :],
        in_offset=bass.IndirectOffsetOnAxis(ap=rids[:, 0:1], axis=0),
    )
    sclv = psmall.tile([P, E], F32, tag="sclv")
    gp.indirect_dma_start(
        out=sclv[:], out_offset=None, in_=scale_dr[:, :],
        in_offset=bass.IndirectOffsetOnAxis(ap=rids[:, 0:1], axis=0),
    )
    ysb = pbig.tile([P, E * D], F32, tag="bigacc")
    for g in range(E):
        gp.indirect_dma_start(
            out=ysb[:, g * D:(g + 1) * D], out_offset=None, in_=eo_out[:, :],
            in_offset=bass.IndirectOffsetOnAxis(ap=sval[:, g:g + 1], axis=0),
        )
    ysb3 = ysb[:].rearrange("p (g d) -> p g d", d=D)
    v.tensor_tensor(out=ysb3, in0=ysb3,
                    in1=sclv[:, :, None].to_broadcast((P, E, D)), op=A.mult)
    sy.dma_start(out=y_d[:, :].rearrange("(p g) d -> p (g d)", p=P), in_=ysb[:])


_BUILD_CACHE = {}


def build_module():
    if "nc" in _BUILD_CACHE:
        return _BUILD_CACHE["nc"]
    nc = bacc.Bacc("TRN2", target_bir_lowering=False, debug=False, num_devices=E)
    io = {
        "x": nc.dram_tensor("x", [T, D], F32, kind="ExternalInput").ap(),
        "xsh": nc.dram_tensor("xsh", [TS, D], F32, kind="ExternalInput").ap(),
        "rwt": nc.dram_tensor("rwt", [P, KD * E], F32, kind="ExternalInput").ap(),
        "rbb": nc.dram_tensor("rbb", [P, E], F32, kind="ExternalInput").ap(),
        "w1h": nc.dram_tensor("w1h", [H, D], F32R, kind="ExternalInput").ap(),
        "b1c": nc.dram_tensor("b1c", [P, MH], F32, kind="ExternalInput").ap(),
        "w2h": nc.dram_tensor("w2h", [H, D], F32R, kind="ExternalInput").ap(),
        "b2c": nc.dram_tensor("b2c", [P, DDN], F32, kind="ExternalInput").ap(),
        "mye": nc.dram_tensor("mye", [P, 1], F32, kind="ExternalInput").ap(),
        "rowids": nc.dram_tensor("rowids", [P, 1], I32, kind="ExternalInput").ap(),
        "y_shard": nc.dram_tensor("y_shard", [TS, D], F32, kind="ExternalOutput").ap(),
        "aux": nc.dram_tensor("aux", [1, 1], F32, kind="ExternalOutput").ap(),
        "counts": nc.dram_tensor("counts", [1, E], I32, kind="ExternalOutput").ap(),
    }
    from contextlib import ExitStack

    with tile.TileContext(nc) as tc:
        with ExitStack() as ctx:
            io["ctx"] = ctx
            _emit(tc, nc, io)
    nc.compile()
    _BUILD_CACHE["nc"] = nc
    return nc


def make_in_maps(x, router_w, router_b, w1, b1, w2, b2):
    x_flat = np.ascontiguousarray(np.asarray(x, np.float32).reshape(T, D))
    rw = np.asarray(router_w, np.float32)
    rb = np.asarray(router_b, np.float32)
    w1 = np.asarray(w1, np.float32)
    b1 = np.asarray(b1, np.float32)
    w2 = np.asarray(w2, np.float32)
    b2 = np.asarray(b2, np.float32)
    rwt = np.ascontiguousarray(rw.reshape(KD, P, E).transpose(1, 0, 2).reshape(P, KD * E))
    rbb = np.ascontiguousarray(np.broadcast_to(rb, (P, E)))
    in_maps = []
    for i in range(E):
        w1h = np.ascontiguousarray(
            w1[i].reshape(KD, P, MH, P).transpose(2, 1, 0, 3).reshape(H, D))
        b1c = np.ascontiguousarray(b1[i].reshape(MH, P).T)
        w2h = np.ascontiguousarray(w2[i])
        b2c = np.ascontiguousarray(b2[i].reshape(DDN, P).T)
        in_maps.append({
            "x": x_flat,
            "xsh": np.ascontiguousarray(x_flat[i * TS:(i + 1) * TS]),
            "rwt": rwt,
            "rbb": rbb,
            "w1h": w1h,
            "b1c": b1c,
            "w2h": w2h,
            "b2c": b2c,
            "mye": np.full((P, 1), float(i), np.float32),
            "rowids": (i * P + np.arange(P, dtype=np.int32)).reshape(P, 1),
        })
    return in_maps


def kernel(x, router_w, router_b, w1, b1, w2, b2):
    from concourse.bass_utils import run_bass_kernel_spmd

    nc = build_module()
    in_maps = make_in_maps(x, router_w, router_b, w1, b1, w2, b2)
    trace = os.environ.get("MOE_TRACE", "0") == "1"
    res = run_bass_kernel_spmd(nc, in_maps, core_ids=list(range(E)), trace=trace)
    if trace and res.exec_time_ns is not None:
        print(f"HW exec time: {res.exec_time_ns} ns")
        _BUILD_CACHE["last_results"] = res
    y = np.concatenate([res.results[i]["y_shard"] for i in range(E)], axis=0)
    y = y.reshape(4, 2048, D)
    aux = np.float32(res.results[0]["aux"].reshape(())[()])
    counts = res.results[0]["counts"].reshape(E).astype(np.int32)
    return y, aux, counts
